# revision 16
# baseline (speedup 1.0000x reference)
"""Trainium2 Bass kernel for MinimalLightningAttention2.

Strategy (8 NeuronCores, SPMD, no collectives):
  core c -> batch b = c // 4, head group g = c % 4 (heads 4g..4g+3).
  Each core computes, fully fused on-chip:
    qkv projection (its 4 heads' columns of Wqkv)
    chunked lightning-attention scan (L=128 chunks, per-head decay state S)
    row-parallel partial of the output projection (its 4 heads' rows of Wout)
  Host sums the 4 partial outputs per batch and adds bout.

Layouts on device (per core):
  xT   [c, it, kt, n]  host-pre-transposed/packed bf16 x, plain DMA
  q,k  [d,   n]  (lhsT = Wq/Wk tile, rhs = xT)
  v    [n, h*d]  (lhsT = xT tile,    rhs = Wv)
  attn output oT [e, i] per head -> directly the lhsT of the Wout matmul.
All matmuls in bf16 (PSUM accumulation fp32); decay masks applied in fp32
during PSUM eviction; decay state S kept in bf16 (rounded once per chunk).

Perf structure (v5, measured ~500-502 us vs 522.8 us for the v1 kernel;
span0's pipeline-fill bubbles are covered by deferring the last v chain
into attention chunk0's slot and splitting outproj(1) across the g2/g3
slots — total PE idle is ~3.2 us over the whole kernel;
PE-matmul floor for this sharding/dtype is ~471 us, fixed NEFF
preamble+barrier ~9 us, so headroom left is ~10 us of DMA-bound startup
ramp, p-state warmup and store-drain tail):
  - startup: x span0 + wq stream in interleaved 256KB (first pieces
    128KB) need-order pieces on the one sync HWDGE ring; q/k projections
    run kt-major (4 parallel PSUM chains) so the weight need-rate matches
    the ring's ~300 GB/s. First matmul at ~10 us, weights fully resident
    by ~47 us.
  - attention is software-pipelined: chunk g's output projection is
    deferred TWO chunks (into chunk g+2's slot, between scores and o
    matmuls), hiding every PSUM-eviction latency across span boundaries;
    eviction engines: DVE for decay/mask/state math, Act for k/outb,
    gpsimd for the S decay premultiply (SBUF-only; gpsimd cannot touch
    PSUM on TRN2).
  - PSUM tags: qk0-3 (bufs=1) shared by q-chains, v-chains and the
    out-projection; sc (bufs=2) by k h0/h1, scores, transposes; o
    (bufs=2) by k h2/h3, o, S-update. All WARs retire >=4 us before
    reuse.
  - mid-kernel stores go on the scalar-engine HWDGE ring (own FIFO,
    never blocks the sync-ring loads); the last three chunks' stores move
    to the sync ring per-512-col-piece so the Act sequencer only runs
    evictions in the tail (DMA issue costs ~1 us of sequencer time each).
  - NOT worth it (measured): XBAR dma transposes instead of PE
    transposes (sequencer-issue cost ~0.9 us each x 16/span dwarfs the
    6.8 us PE saving, and non-contiguous destinations corrupted data on
    HW); fp8 DoubleRow (needs a 3-term split for the 2e-2 error gate ->
    1.5x bf16 time).
"""

import math

import numpy as np
import ml_dtypes

B, N, C = 2, 4096, 2048
H_TOT = 16
HD = 128          # head dim
H = 4             # heads per core
L = 128           # attention chunk length
KT = C // 128     # 16 contraction tiles for the projections
NSPAN = 512       # tokens per outer iteration
NIT = N // NSPAN  # 8 outer iterations
P = 128

BF16 = ml_dtypes.bfloat16

_CACHE = {}


def _build():
    """Build + compile the SPMD Bass program (same program on all 8 cores)."""
    from contextlib import ExitStack

    import concourse.bass as bass
    import concourse.tile as tile
    from concourse import bacc, mybir

    DT = mybir.dt.bfloat16
    F32 = mybir.dt.float32

    nc = bacc.Bacc(
        "TRN2",
        target_bir_lowering=False,
        debug=False,
        enable_asserts=False,
        num_devices=8,
    )

    # host-packed transpose of x: x[c, it, kt, n] = x[it*512+n, kt*128+c]
    xd = nc.dram_tensor("x", [P, NIT, KT, NSPAN], DT, kind="ExternalInput").ap()
    # host-packed: [c, kt*512 + col] (col = head*128 + d), contiguous rows
    wqd = nc.dram_tensor("wq", [P, KT * 512], DT, kind="ExternalInput").ap()
    wkd = nc.dram_tensor("wk", [P, KT * 512], DT, kind="ExternalInput").ap()
    wvd = nc.dram_tensor("wv", [P, KT * 512], DT, kind="ExternalInput").ap()
    # host-packed: [d, ct*2048 + h*512 + outc]
    wod = nc.dram_tensor("wo", [P, 4 * H * 512], DT, kind="ExternalInput").ap()
    masktd = nc.dram_tensor("maskt", [P, H * L], F32, kind="ExternalInput").ap()
    qdecd = nc.dram_tensor("qdec", [P, H * NSPAN], F32, kind="ExternalInput").ap()
    kdecvd = nc.dram_tensor("kdecv", [P, H * HD], F32, kind="ExternalInput").ap()
    bdfd = nc.dram_tensor("bdf", [P, H * HD], F32, kind="ExternalInput").ap()
    bqkd = nc.dram_tensor("bqk", [P, 2 * H], F32, kind="ExternalInput").ap()
    bvfd = nc.dram_tensor("bvf", [P, H * HD], F32, kind="ExternalInput").ap()
    # bf16 partial-output stores: halves store traffic (the host sums the
    # four per-batch partials in fp32; bf16 partial rounding adds ~0.1-0.3%
    # norm error vs the 2e-2 gate)
    outd = nc.dram_tensor("out", [N, C], DT, kind="ExternalOutput").ap()

    mult = mybir.AluOpType.mult
    add = mybir.AluOpType.add
    IDENT = mybir.ActivationFunctionType.Identity

    with tile.TileContext(nc) as tc:
        with ExitStack() as ctx:
            const = ctx.enter_context(tc.tile_pool(name="const", bufs=1))
            xt_pool = ctx.enter_context(tc.tile_pool(name="xt", bufs=1))
            qk_pool = ctx.enter_context(tc.tile_pool(name="qk", bufs=1))
            sb_pool = ctx.enter_context(tc.tile_pool(name="sb", bufs=2))
            outb_pool = ctx.enter_context(tc.tile_pool(name="outb", bufs=2))
            ps = ctx.enter_context(tc.tile_pool(name="ps", bufs=1, space="PSUM"))

            # ---- startup loads: TWO HWDGE rings, exact need-order ----
            # The rings share HBM bandwidth (~350 GB/s aggregate, ~175
            # each when both stream), so the split is deadline-balanced:
            #   sync ring:   x span0, q biases, qdec, decay consts, wo,
            #                x span1   (8 MB)
            #   scalar ring: wq, wk, wv (6 MB) - the Act engine is idle
            #                until the first k eviction and its ring FIFO
            #                drains before the first mid-kernel store.
            # With x and wq on separate rings the q projection's ~294 GB/s
            # combined need-rate is met from the first piece on, instead
            # of starving for ~2.8us behind a single ~300 GB/s ring.
            xp0 = []
            wq_p = []
            for j in range(8):
                xq = xt_pool.tile([P, 2, NSPAN], DT, tag=f"xp{j}", name=f"xp0_{j}")
                wt = const.tile([P, 1024], DT, tag=f"wqp{j}", name=f"wq_{j}")
                if j == 0:
                    # first pieces at 1-kt granularity so the PE starts ASAP
                    for q in range(2):
                        nc.sync.dma_start(xq[:, q, :], xd[:, 0, q, :])
                        nc.scalar.dma_start(wt[:, q * 512:(q + 1) * 512],
                                            wqd[:, q * 512:(q + 1) * 512])
                else:
                    nc.sync.dma_start(xq[:], xd[:, 0, 2 * j:2 * j + 2, :])
                    nc.scalar.dma_start(wt[:], wqd[:, j * 1024:(j + 1) * 1024])
                xp0.append(xq)
                wq_p.append(wt)
            wk_p = []
            for j in range(4):
                wt = const.tile([P, 2048], DT, tag=f"wkp{j}", name=f"wk_{j}")
                nc.scalar.dma_start(wt[:], wkd[:, j * 2048:(j + 1) * 2048])
                wk_p.append(wt)
            bqk_sb = const.tile([P, 2 * H], F32)
            nc.sync.dma_start(bqk_sb[:], bqkd[:])
            qdec_sb = const.tile([P, H * NSPAN], F32)
            nc.sync.dma_start(qdec_sb[:], qdecd[:])
            maskt_sb = const.tile([P, H * L], F32)
            nc.sync.dma_start(maskt_sb[:], masktd[:])
            kdecv_sb = const.tile([P, H * HD], F32)
            nc.sync.dma_start(kdecv_sb[:], kdecvd[:])
            bdf_sb = const.tile([P, H * HD], F32)
            nc.sync.dma_start(bdf_sb[:], bdfd[:])
            bvf_sb = const.tile([P, H * HD], F32)
            nc.sync.dma_start(bvf_sb[:], bvfd[:])
            wv_p = []
            for j in range(4):
                wt = const.tile([P, 2048], DT, tag=f"wvp{j}", name=f"wv_{j}")
                nc.scalar.dma_start(wt[:], wvd[:, j * 2048:(j + 1) * 2048])
                wv_p.append(wt)
            wo_p = []
            for ct in range(4):
                wt = const.tile([P, H * 512], DT, tag=f"wop{ct}", name=f"wo_{ct}")
                nc.sync.dma_start(wt[:], wod[:, ct * 2048:(ct + 1) * 2048])
                wo_p.append(wt)
            xs_tiles = {}
            xs1 = xt_pool.tile([P, KT, NSPAN], DT, tag="xs", bufs=2, name="xs1")
            nc.sync.dma_start(xs1[:], xd[:, 1, :, :])
            xs_tiles[1] = xs1

            ident = const.tile([P, P], DT)
            from concourse.masks import make_identity
            make_identity(nc, ident)

            # PE clock warm-up: the tensor engine ramps 0.65->2.4GHz over
            # ~3us of continuous execution.  It would otherwise sit idle
            # until the first x/wq pieces land (~10us), then pay the ramp
            # on real work; instead run throwaway matmuls on the identity
            # tile so the q chains start at (nearly) full clock.  The tag-o
            # bank's first real user (k h2) comes ~25us later.
            warm_ps = ps.tile([P, 512], F32, tag="o", bufs=2, name="warm_ps")
            for i in range(32):
                nc.tensor.matmul(
                    warm_ps[:, (i % 4) * P:(i % 4 + 1) * P],
                    lhsT=ident[:], rhs=ident[:], start=True, stop=True,
                )

            # per-head decay state S [d, e], 4 heads side by side, bf16
            S_bf = const.tile([P, H * HD], DT)
            nc.vector.memset(S_bf[:], 0.0)
            tmp_sb = const.tile([P, H * HD], F32)

            def xts(it, kt):
                if it == 0:
                    return xp0[kt // 2][:, kt % 2, :]
                return xs_tiles[it][:, kt, :]

            def wq_ap(kt, h):
                return wq_p[kt // 2][:, (kt % 2) * 512 + h * HD:(kt % 2) * 512 + (h + 1) * HD]

            def wk_ap(kt, h):
                return wk_p[kt // 4][:, (kt % 4) * 512 + h * HD:(kt % 4) * 512 + (h + 1) * HD]

            def wv_ap(kt):
                return wv_p[kt // 4][:, (kt % 4) * 512:(kt % 4 + 1) * 512]

            # deferred output projections, emitted TWO chunks late so the
            # ob eviction has a ~5us window instead of 0.6us
            pending = []
            warm_outb = [None]

            def emit_outproj(ob, g, mode="norm", cts=(0, 1, 2, 3), outb=None):
                n0 = g * L
                if outb is None:
                    outb = outb_pool.tile([P, C], DT, tag="outb", name=f"outb{g}")
                for ct in cts:
                    ops = ps.tile([P, 512], F32, tag=f"qk{ct}", bufs=1,
                                  name=f"outps{g}_{ct}")
                    for h in range(H):
                        nc.tensor.matmul(
                            ops[:],
                            lhsT=ob[:, h * L:(h + 1) * L],
                            rhs=wo_p[ct][:, h * 512:(h + 1) * 512],
                            start=(h == 0), stop=(h == H - 1),
                        )
                    c0 = ct * 512
                    if mode == "norm":
                        nc.scalar.copy(outb[:, c0:c0 + 512], ops[:])
                    else:
                        # epilogue: split halves across DVE+Act so neither
                        # sequencer serializes the tail, store per-ct pieces
                        # as soon as each is ready
                        nc.vector.tensor_copy(outb[:, c0:c0 + 256], ops[:, 0:256])
                        nc.scalar.copy(outb[:, c0 + 256:c0 + 512], ops[:, 256:512])
                        nc.sync.dma_start(
                            outd[n0:n0 + L, c0:c0 + 512], outb[:, c0:c0 + 512])
                if mode == "norm" and 3 in cts:
                    # all norm-mode stores on the Act ring: the epilogue's
                    # stores run on the sync ring, so chunk 28/29's 0.5MB
                    # stores must NOT queue ahead of them there
                    nc.scalar.dma_start(outd[n0:n0 + L, :], outb[:])
                return outb

            def emit_attn_chunk(it, p, q_raw, q_dec, k_sb, v_sb, kTs, filler=None):
                g = it * 4 + p
                # scoresT for all 4 heads into one psum bank (before vdec:
                # o waits on scT, so its eviction goes first)
                sc_ps = ps.tile([P, 512], F32, tag="sc", bufs=2, name=f"scps{g}")
                for h in range(H):
                    nc.tensor.matmul(
                        sc_ps[:, h * L:(h + 1) * L],
                        lhsT=k_sb[:, h * NSPAN + p * L: h * NSPAN + (p + 1) * L],
                        rhs=q_raw[:, h * NSPAN + p * L: h * NSPAN + (p + 1) * L],
                        start=True, stop=True,
                    )
                scT = sb_pool.tile([P, 512], DT, tag="scT", name=f"scT{g}")
                nc.vector.tensor_tensor(scT[:], sc_ps[:], maskt_sb[:], op=mult)

                # fold the per-token k decay into a decayed copy of v (the
                # j-major k copy now comes from the XBAR dma transpose, so
                # there is no kT eviction to fold it into); same DVE cost
                # as the old kT eviction multiply
                vdec = sb_pool.tile([P, 512], DT, tag="vdec", name=f"vdec{g}")
                nc.vector.tensor_tensor(
                    vdec[:], v_sb[:, p * 512:(p + 1) * 512], kdecv_sb[:], op=mult)

                # a deferred output projection runs here, covering the
                # scT/S_bf eviction latencies with 3.4us of PE work.
                # Pipeline fill (span0 only): chunk0's slot takes the
                # deferred v(ns3) chain, g==1 runs defer-1, and outproj(1)
                # splits across the g==2/g==3 slots, so every early chunk
                # has PE cover while transitioning to steady defer-2.
                if filler is not None:
                    filler()
                if g == 1 and pending:
                    emit_outproj(*pending.pop(0))
                elif g == 2 and pending:
                    warm_outb[0] = emit_outproj(*pending[0], cts=(0, 1))
                elif g == 3 and len(pending) >= 2:
                    ob0, g0 = pending.pop(0)
                    emit_outproj(ob0, g0, cts=(2, 3), outb=warm_outb[0])
                elif len(pending) >= 2:
                    emit_outproj(*pending.pop(0))

                # o = v^T @ scoresT + S^T @ qdec   [e, i] per head
                o_ps = ps.tile([P, 512], F32, tag="o", bufs=2, name=f"ops{g}")
                for h in range(H):
                    nc.tensor.matmul(
                        o_ps[:, h * L:(h + 1) * L],
                        lhsT=v_sb[:, p * 512 + h * HD: p * 512 + (h + 1) * HD],
                        rhs=scT[:, h * L:(h + 1) * L],
                        start=True, stop=False,
                    )
                    nc.tensor.matmul(
                        o_ps[:, h * L:(h + 1) * L],
                        lhsT=S_bf[:, h * HD:(h + 1) * HD],
                        rhs=q_dec[:, h * NSPAN + p * L: h * NSPAN + (p + 1) * L],
                        start=False, stop=True,
                    )
                ob = sb_pool.tile([P, 512], DT, tag="ob", bufs=3, name=f"ob{g}")
                nc.vector.tensor_copy(ob[:, 0:256], o_ps[:, 0:256])
                nc.scalar.copy(ob[:, 256:512], o_ps[:, 256:512])

                # S <- S * blockdecay + kT^T @ (kdec * v)
                su_ps = ps.tile([P, 512], F32, tag="o", bufs=2, name=f"sups{g}")
                for h in range(H):
                    nc.tensor.matmul(
                        su_ps[:, h * HD:(h + 1) * HD],
                        lhsT=kTs[:, h * 4 + p, :],
                        rhs=vdec[:, h * HD:(h + 1) * HD],
                        start=True, stop=True,
                    )
                # S decay premultiply on gpsimd (SBUF->SBUF): keeps DVE free
                nc.gpsimd.tensor_tensor(tmp_sb[:], S_bf[:], bdf_sb[:], op=mult)
                nc.vector.tensor_tensor(S_bf[:], tmp_sb[:], su_ps[:], op=add)
                pending.append((ob, g))

            for it in range(NIT):
                if 1 <= it < NIT - 1:
                    xs = xt_pool.tile([P, KT, NSPAN], DT, tag="xs", bufs=2,
                                      name=f"xs{it + 1}")
                    nc.sync.dma_start(xs[:], xd[:, it + 1, :, :])
                    xs_tiles[it + 1] = xs

                # ---- q projection, kt-major (4 parallel PSUM chains) ----
                q_ps_l = [ps.tile([P, NSPAN], F32, tag=f"qk{h}", bufs=1,
                                  name=f"qps{it}_{h}") for h in range(H)]
                for kt in range(KT):
                    for h in range(H):
                        nc.tensor.matmul(
                            q_ps_l[h][:], lhsT=wq_ap(kt, h), rhs=xts(it, kt),
                            start=(kt == 0), stop=(kt == KT - 1),
                        )
                q_raw = qk_pool.tile([P, H * NSPAN], DT, tag="q_raw", name=f"q_raw{it}")
                for h in range(H):
                    nc.vector.tensor_scalar_add(
                        q_raw[:, h * NSPAN:(h + 1) * NSPAN], q_ps_l[h][:],
                        bqk_sb[:, 2 * h:2 * h + 1])
                # q_dec from SBUF q_raw (frees the psum bank after ONE eviction)
                q_dec = qk_pool.tile([P, H * NSPAN], DT, tag="q_dec", name=f"q_dec{it}")
                for h in range(H):
                    hs = slice(h * NSPAN, (h + 1) * NSPAN)
                    nc.vector.tensor_tensor(q_dec[:, hs], q_raw[:, hs],
                                            qdec_sb[:, hs], op=mult)

                # ---- k projection, kt-major into the attn banks ----
                k_ps_l = [ps.tile([P, NSPAN], F32, tag=("sc" if h < 2 else "o"),
                                  bufs=2, name=f"kps{it}_{h}") for h in range(H)]
                for kt in range(KT):
                    for h in range(H):
                        nc.tensor.matmul(
                            k_ps_l[h][:], lhsT=wk_ap(kt, h), rhs=xts(it, kt),
                            start=(kt == 0), stop=(kt == KT - 1),
                        )
                k_sb = qk_pool.tile([P, H * NSPAN], DT, tag="k_sb", name=f"k_sb{it}")
                for h in range(H):
                    nc.scalar.activation(
                        k_sb[:, h * NSPAN:(h + 1) * NSPAN], k_ps_l[h][:],
                        IDENT, bias=bqk_sb[:, 2 * h + 1:2 * h + 2])

                # j-major k for the S updates: ONE batched XBAR dma
                # transpose of all 16 [d,128]-blocks of k_sb (~3.6us DMA,
                # one sync-ring issue) replaces 16 PE transposes (~0.85us
                # of PE per span).  kTs[j, h*4+p, d] = k_sb[d, h*512+p*128+j].
                kTs = qk_pool.tile([P, H * 4, HD], DT, tag="kTs", name=f"kTs{it}")
                nc.sync.dma_start_transpose(kTs[:], k_sb[:])


                # ---- v projection ----
                v_sb = qk_pool.tile([P, H * NSPAN], DT, tag="v_sb", name=f"v_sb{it}")

                def v_chain(ns, it=it, v_sb=v_sb):
                    vps = ps.tile([P, 512], F32, tag=f"qk{ns}", bufs=1,
                                  name=f"vps{it}_{ns}")
                    for kt in range(KT):
                        nc.tensor.matmul(
                            vps[:], lhsT=xts(it, kt)[:, ns * P:(ns + 1) * P],
                            rhs=wv_ap(kt), start=(kt == 0), stop=(kt == KT - 1),
                        )
                    nc.vector.tensor_tensor(
                        v_sb[:, ns * 512:(ns + 1) * 512], vps[:], bvf_sb[:], op=add)

                # span0 has no deferred outproj to cover chunk0's scT/kT
                # eviction latencies, so its last v chain (only needed by
                # chunk 3) moves into chunk0's outproj slot as PE filler
                for ns in range(3 if it == 0 else 4):
                    v_chain(ns)

                # ---- attention + (deferred) output projection ----
                for p in range(4):
                    filler = (lambda: v_chain(3)) if (it == 0 and p == 0) else None
                    emit_attn_chunk(it, p, q_raw, q_dec, k_sb, v_sb, kTs, filler)

            # epilogue: remaining deferred output projections, minimal tail
            emit_outproj(*pending.pop(0), mode="pen")
            emit_outproj(*pending.pop(0), mode="last")

    nc.compile()
    return nc


def _host_inputs(x, Wqkv, bqkv, Wout, bout, slopes):
    """Per-core input maps (numpy, host-side sharding + packing)."""
    in_maps = []
    # packed transpose of x, shared by the 4 cores of each batch:
    # xp[c, it, kt, n] = x[b, it*512+n, kt*128+c]
    _xtp_cache = [
        np.ascontiguousarray(
            x[b].astype(BF16).reshape(NIT, NSPAN, KT, P).transpose(3, 0, 2, 1)
        )
        for b in range(B)
    ]
    i = np.arange(L, dtype=np.float64)
    for core in range(8):
        b, g = core // 4, core % 4
        h0 = 4 * g
        hsel = slice(h0 * HD, (h0 + H) * HD)

        xb = _xtp_cache[b]

        def pack_w(Wslice):
            # (C, 512) -> [c_in_tile(128), kt*512 + col]
            return np.ascontiguousarray(
                Wslice.astype(BF16).reshape(KT, P, H * HD).transpose(1, 0, 2).reshape(P, KT * 512)
            )

        wq = pack_w(Wqkv[:, 0 * C:1 * C][:, hsel])
        wk = pack_w(Wqkv[:, 1 * C:2 * C][:, hsel])
        wv = pack_w(Wqkv[:, 2 * C:3 * C][:, hsel])
        # Wout rows for these heads: [d(128), ct*2048 + h*512 + outc]
        wo = np.ascontiguousarray(
            Wout[hsel, :].astype(BF16).reshape(H, HD, 4, 512)
            .transpose(1, 2, 0, 3).reshape(P, 4 * H * 512)
        )

        s = slopes[h0:h0 + H].astype(np.float64)  # (4,)
        diffT = (i[None, :] - i[:, None])          # [j, i] = i - j
        maskt = np.concatenate(
            [np.where(diffT >= 0, np.exp(-s[h] * diffT), 0.0) for h in range(H)],
            axis=1,
        ).astype(np.float32)                       # [128, 4*128]
        qdec_l = [np.exp(-s[h] * i) for h in range(H)]        # each (L,)
        qdec = np.concatenate(
            [np.broadcast_to(np.tile(qdec_l[h], NSPAN // L)[None, :], (P, NSPAN)) for h in range(H)],
            axis=1,
        ).astype(np.float32)                       # [128, 4*512]
        kdecv = np.concatenate(
            [np.broadcast_to(np.exp(-s[h] * (L - i))[:, None], (P, HD)) for h in range(H)],
            axis=1,
        ).astype(np.float32)                       # [128, 4*128]
        bdf = np.concatenate(
            [np.full((P, HD), math.exp(-s[h] * L)) for h in range(H)], axis=1
        ).astype(np.float32)
        # per-head, per-partition(d) q/k biases: columns [bq_h0, bk_h0, bq_h1, ...]
        bq_heads = bqkv[0 * C:1 * C][hsel].reshape(H, HD)
        bk_heads = bqkv[1 * C:2 * C][hsel].reshape(H, HD)
        bqk = np.zeros((P, 2 * H), dtype=np.float32)
        for h in range(H):
            bqk[:, 2 * h] = bq_heads[h]
            bqk[:, 2 * h + 1] = bk_heads[h]
        bvf = np.broadcast_to(bqkv[2 * C:3 * C][hsel][None, :], (P, H * HD)).astype(np.float32)

        in_maps.append({
            "x": xb, "wq": wq, "wk": wk, "wv": wv, "wo": wo,
            "maskt": maskt, "qdec": qdec, "kdecv": kdecv, "bdf": bdf,
            "bqk": bqk, "bvf": np.ascontiguousarray(bvf),
        })
    return in_maps


def kernel(x, Wqkv, bqkv, Wout, bout, slopes, _want_trace=False):
    from concourse import bass_utils

    x = np.asarray(x, dtype=np.float32)
    Wqkv = np.asarray(Wqkv, dtype=np.float32)
    bqkv = np.asarray(bqkv, dtype=np.float32)
    Wout = np.asarray(Wout, dtype=np.float32)
    bout = np.asarray(bout, dtype=np.float32)
    slopes = np.asarray(slopes, dtype=np.float32)

    if "nc" not in _CACHE:
        _CACHE["nc"] = _build()
    nc = _CACHE["nc"]

    in_maps = _host_inputs(x, Wqkv, bqkv, Wout, bout, slopes)
    res = bass_utils.run_bass_kernel_spmd(
        nc, in_maps, core_ids=list(range(8)), trace=_want_trace,
    )
    out = np.zeros((B, N, C), dtype=np.float32)
    for core in range(8):
        out[core // 4] += res.results[core]["out"].astype(np.float32)
    out += bout[None, None, :]
    if _want_trace:
        _CACHE["last_result"] = res
    return out



# revision 18
# speedup vs baseline: 1.0131x; 1.0131x over previous
"""Trainium2 Bass kernel for MinimalLightningAttention2.

Strategy (8 NeuronCores, SPMD, no collectives):
  core c -> batch b = c // 4, head group g = c % 4 (heads 4g..4g+3).
  Each core computes, fully fused on-chip:
    qkv projection (its 4 heads' columns of Wqkv)
    chunked lightning-attention scan (L=128 chunks, per-head decay state S)
    row-parallel partial of the output projection (its 4 heads' rows of Wout)
  Host sums the 4 partial outputs per batch and adds bout.

Layouts on device (per core):
  xT   [c, it, kt, n]  host-pre-transposed/packed bf16 x, plain DMA
  q,k  [d,   n]  (lhsT = Wq/Wk tile, rhs = xT)
  v    [n, h*d]  (lhsT = xT tile,    rhs = Wv)
  attn output oT [e, i] per head -> directly the lhsT of the Wout matmul.
All matmuls in bf16 (PSUM accumulation fp32); decay masks applied in fp32
during PSUM eviction; decay state S kept in bf16 (rounded once per chunk).

Perf structure (v5, measured ~500-502 us vs 522.8 us for the v1 kernel;
span0's pipeline-fill bubbles are covered by deferring the last v chain
into attention chunk0's slot and splitting outproj(1) across the g2/g3
slots — total PE idle is ~3.2 us over the whole kernel;
PE-matmul floor for this sharding/dtype is ~471 us, fixed NEFF
preamble+barrier ~9 us, so headroom left is ~10 us of DMA-bound startup
ramp, p-state warmup and store-drain tail):
  - startup: x span0 + wq stream in interleaved 256KB (first pieces
    128KB) need-order pieces on the one sync HWDGE ring; q/k projections
    run kt-major (4 parallel PSUM chains) so the weight need-rate matches
    the ring's ~300 GB/s. First matmul at ~10 us, weights fully resident
    by ~47 us.
  - attention is software-pipelined: chunk g's output projection is
    deferred TWO chunks (into chunk g+2's slot, between scores and o
    matmuls), hiding every PSUM-eviction latency across span boundaries;
    eviction engines: DVE for decay/mask/state math, Act for k/outb,
    gpsimd for the S decay premultiply (SBUF-only; gpsimd cannot touch
    PSUM on TRN2).
  - PSUM tags: qk0-3 (bufs=1) shared by q-chains, v-chains and the
    out-projection; sc (bufs=2) by k h0/h1, scores, transposes; o
    (bufs=2) by k h2/h3, o, S-update. All WARs retire >=4 us before
    reuse.
  - mid-kernel stores go on the scalar-engine HWDGE ring (own FIFO,
    never blocks the sync-ring loads); the last three chunks' stores move
    to the sync ring per-512-col-piece so the Act sequencer only runs
    evictions in the tail (DMA issue costs ~1 us of sequencer time each).
  - NOT worth it (measured): XBAR dma transposes instead of PE
    transposes (sequencer-issue cost ~0.9 us each x 16/span dwarfs the
    6.8 us PE saving, and non-contiguous destinations corrupted data on
    HW); fp8 DoubleRow (needs a 3-term split for the 2e-2 error gate ->
    1.5x bf16 time).
"""

import math

import numpy as np
import ml_dtypes

B, N, C = 2, 4096, 2048
H_TOT = 16
HD = 128          # head dim
H = 4             # heads per core
L = 128           # attention chunk length
KT = C // 128     # 16 contraction tiles for the projections
NSPAN = 512       # tokens per outer iteration
NIT = N // NSPAN  # 8 outer iterations
P = 128

BF16 = ml_dtypes.bfloat16

_CACHE = {}


def _build():
    """Build + compile the SPMD Bass program (same program on all 8 cores)."""
    from contextlib import ExitStack

    import concourse.bass as bass
    import concourse.tile as tile
    from concourse import bacc, mybir

    DT = mybir.dt.bfloat16
    F32 = mybir.dt.float32

    nc = bacc.Bacc(
        "TRN2",
        target_bir_lowering=False,
        debug=False,
        enable_asserts=False,
        num_devices=8,
    )

    # host-packed transpose of x: x[c, it, kt, n] = x[it*512+n, kt*128+c]
    xd = nc.dram_tensor("x", [P, NIT, KT, NSPAN], DT, kind="ExternalInput").ap()
    # host-packed: [c, kt*512 + col] (col = head*128 + d), contiguous rows
    wqd = nc.dram_tensor("wq", [P, KT * 512], DT, kind="ExternalInput").ap()
    wkd = nc.dram_tensor("wk", [P, KT * 512], DT, kind="ExternalInput").ap()
    wvd = nc.dram_tensor("wv", [P, KT * 512], DT, kind="ExternalInput").ap()
    # host-packed: [d, ct*2048 + h*512 + outc]
    wod = nc.dram_tensor("wo", [P, 4 * H * 512], DT, kind="ExternalInput").ap()
    masktd = nc.dram_tensor("maskt", [P, H * L], F32, kind="ExternalInput").ap()
    qdecd = nc.dram_tensor("qdec", [P, H * NSPAN], F32, kind="ExternalInput").ap()
    kdecvd = nc.dram_tensor("kdecv", [P, H * HD], F32, kind="ExternalInput").ap()
    bdfd = nc.dram_tensor("bdf", [P, H * HD], F32, kind="ExternalInput").ap()
    bqkd = nc.dram_tensor("bqk", [P, 2 * H], F32, kind="ExternalInput").ap()
    bvfd = nc.dram_tensor("bvf", [P, H * HD], F32, kind="ExternalInput").ap()
    # bf16 partial-output stores: halves store traffic (the host sums the
    # four per-batch partials in fp32; bf16 partial rounding adds ~0.1-0.3%
    # norm error vs the 2e-2 gate)
    outd = nc.dram_tensor("out", [N, C], DT, kind="ExternalOutput").ap()

    mult = mybir.AluOpType.mult
    add = mybir.AluOpType.add
    IDENT = mybir.ActivationFunctionType.Identity

    with tile.TileContext(nc) as tc:
        with ExitStack() as ctx:
            const = ctx.enter_context(tc.tile_pool(name="const", bufs=1))
            xt_pool = ctx.enter_context(tc.tile_pool(name="xt", bufs=1))
            qk_pool = ctx.enter_context(tc.tile_pool(name="qk", bufs=1))
            sb_pool = ctx.enter_context(tc.tile_pool(name="sb", bufs=2))
            outb_pool = ctx.enter_context(tc.tile_pool(name="outb", bufs=2))
            ps = ctx.enter_context(tc.tile_pool(name="ps", bufs=1, space="PSUM"))

            # ---- startup loads: ONE sync HWDGE ring, exact need-order ----
            # The per-core HBM read path caps at ~300 GB/s TOTAL (measured:
            # splitting the stream across the sync+scalar rings does not
            # increase aggregate bandwidth, it only reorders arrivals), so
            # everything goes on the sync ring in exact need order:
            # x span0 + wq interleaved in 256KB pieces, wk, the small
            # decay/bias consts, wv, qdec, x span1, wo.
            xp0 = []
            wq_p = []
            for j in range(8):
                xq = xt_pool.tile([P, 2, NSPAN], DT, tag=f"xp{j}", name=f"xp0_{j}")
                wt = const.tile([P, 1024], DT, tag=f"wqp{j}", name=f"wq_{j}")
                if j == 0:
                    # first pieces at 1-kt granularity so the PE starts ASAP
                    for q in range(2):
                        nc.sync.dma_start(xq[:, q, :], xd[:, 0, q, :])
                        nc.sync.dma_start(wt[:, q * 512:(q + 1) * 512],
                                          wqd[:, q * 512:(q + 1) * 512])
                else:
                    nc.sync.dma_start(xq[:], xd[:, 0, 2 * j:2 * j + 2, :])
                    nc.sync.dma_start(wt[:], wqd[:, j * 1024:(j + 1) * 1024])
                xp0.append(xq)
                wq_p.append(wt)
            wk_p = []
            for j in range(4):
                wt = const.tile([P, 2048], DT, tag=f"wkp{j}", name=f"wk_{j}")
                nc.sync.dma_start(wt[:], wkd[:, j * 2048:(j + 1) * 2048])
                wk_p.append(wt)
            bqk_sb = const.tile([P, 2 * H], F32)
            nc.sync.dma_start(bqk_sb[:], bqkd[:])
            maskt_sb = const.tile([P, H * L], F32)
            nc.sync.dma_start(maskt_sb[:], masktd[:])
            kdecv_sb = const.tile([P, H * HD], F32)
            nc.sync.dma_start(kdecv_sb[:], kdecvd[:])
            bdf_sb = const.tile([P, H * HD], F32)
            nc.sync.dma_start(bdf_sb[:], bdfd[:])
            bvf_sb = const.tile([P, H * HD], F32)
            nc.sync.dma_start(bvf_sb[:], bvfd[:])
            wv_p = []
            for j in range(4):
                wt = const.tile([P, 2048], DT, tag=f"wvp{j}", name=f"wv_{j}")
                nc.sync.dma_start(wt[:], wvd[:, j * 2048:(j + 1) * 2048])
                wv_p.append(wt)
            qdec_sb = const.tile([P, H * NSPAN], F32)
            nc.sync.dma_start(qdec_sb[:], qdecd[:])
            # span-1 x before wo: needed at ~56us, wo (defer-2) only at ~58us
            xs_tiles = {}
            xs1 = xt_pool.tile([P, KT, NSPAN], DT, tag="xs", bufs=2, name="xs1")
            nc.sync.dma_start(xs1[:], xd[:, 1, :, :])
            xs_tiles[1] = xs1
            wo_p = []
            for ct in range(4):
                wt = const.tile([P, H * 512], DT, tag=f"wop{ct}", name=f"wo_{ct}")
                nc.sync.dma_start(wt[:], wod[:, ct * 2048:(ct + 1) * 2048])
                wo_p.append(wt)

            # PE clock warm-up: the tensor engine runs at 1.2GHz until HAM
            # sees ~3.4us of sustained activity.  It would otherwise sit
            # idle until the first x/wq pieces land (~7-10us), then pay the
            # ramp on real work; instead run throwaway matmuls ASAP.  The
            # operand values are irrelevant, so a DVE memset tile (ready
            # ~5us, vs ~7.3us for the old gpsimd-iota identity) feeds them.
            # The tag-o bank's first real user (k h2) comes ~25us later.
            warm_in = const.tile([P, P], DT)
            nc.vector.memset(warm_in[:], 0.0)
            warm_ps = ps.tile([P, 512], F32, tag="o", bufs=2, name="warm_ps")
            for i in range(24):
                nc.tensor.matmul(
                    warm_ps[:, (i % 4) * P:(i % 4 + 1) * P],
                    lhsT=warm_in[:], rhs=warm_in[:], start=True, stop=True,
                )

            # per-head decay state S [d, e], 4 heads side by side, bf16
            S_bf = const.tile([P, H * HD], DT)
            nc.vector.memset(S_bf[:], 0.0)
            tmp_sb = const.tile([P, H * HD], F32)

            def xts(it, kt):
                if it == 0:
                    return xp0[kt // 2][:, kt % 2, :]
                return xs_tiles[it][:, kt, :]

            def wq_ap(kt, h):
                return wq_p[kt // 2][:, (kt % 2) * 512 + h * HD:(kt % 2) * 512 + (h + 1) * HD]

            def wk_ap(kt, h):
                return wk_p[kt // 4][:, (kt % 4) * 512 + h * HD:(kt % 4) * 512 + (h + 1) * HD]

            def wv_ap(kt):
                return wv_p[kt // 4][:, (kt % 4) * 512:(kt % 4 + 1) * 512]

            # deferred output projections, emitted TWO chunks late so the
            # ob eviction has a ~5us window instead of 0.6us
            pending = []
            warm_outb = [None]

            def emit_outproj(ob, g, mode="norm", cts=(0, 1, 2, 3), outb=None):
                n0 = g * L
                if outb is None:
                    outb = outb_pool.tile([P, C], DT, tag="outb", name=f"outb{g}")
                for ct in cts:
                    ops = ps.tile([P, 512], F32, tag=f"qk{ct}", bufs=1,
                                  name=f"outps{g}_{ct}")
                    for h in range(H):
                        nc.tensor.matmul(
                            ops[:],
                            lhsT=ob[:, h * L:(h + 1) * L],
                            rhs=wo_p[ct][:, h * 512:(h + 1) * 512],
                            start=(h == 0), stop=(h == H - 1),
                        )
                    c0 = ct * 512
                    if mode == "norm":
                        nc.scalar.copy(outb[:, c0:c0 + 512], ops[:])
                    else:
                        # epilogue: split halves across DVE+Act so neither
                        # sequencer serializes the tail, store per-ct pieces
                        # as soon as each is ready
                        nc.vector.tensor_copy(outb[:, c0:c0 + 256], ops[:, 0:256])
                        nc.scalar.copy(outb[:, c0 + 256:c0 + 512], ops[:, 256:512])
                        nc.sync.dma_start(
                            outd[n0:n0 + L, c0:c0 + 512], outb[:, c0:c0 + 512])
                if mode == "norm" and 3 in cts:
                    # all norm-mode stores on the Act ring: the epilogue's
                    # stores run on the sync ring, so chunk 28/29's 0.5MB
                    # stores must NOT queue ahead of them there
                    nc.scalar.dma_start(outd[n0:n0 + L, :], outb[:])
                return outb

            def emit_attn_chunk(it, p, q_raw, q_dec, k_sb, v_sb, kTs, filler=None):
                g = it * 4 + p
                # scoresT for all 4 heads into one psum bank (before vdec:
                # o waits on scT, so its eviction goes first)
                sc_ps = ps.tile([P, 512], F32, tag="sc", bufs=2, name=f"scps{g}")
                for h in range(H):
                    nc.tensor.matmul(
                        sc_ps[:, h * L:(h + 1) * L],
                        lhsT=k_sb[:, h * NSPAN + p * L: h * NSPAN + (p + 1) * L],
                        rhs=q_raw[:, h * NSPAN + p * L: h * NSPAN + (p + 1) * L],
                        start=True, stop=True,
                    )
                scT = sb_pool.tile([P, 512], DT, tag="scT", name=f"scT{g}")
                nc.vector.tensor_tensor(scT[:], sc_ps[:], maskt_sb[:], op=mult)

                # fold the per-token k decay into a decayed copy of v (the
                # j-major k copy now comes from the XBAR dma transpose, so
                # there is no kT eviction to fold it into); same DVE cost
                # as the old kT eviction multiply
                vdec = sb_pool.tile([P, 512], DT, tag="vdec", name=f"vdec{g}")
                nc.vector.tensor_tensor(
                    vdec[:], v_sb[:, p * 512:(p + 1) * 512], kdecv_sb[:], op=mult)

                # a deferred output projection runs here, covering the
                # scT/S_bf eviction latencies with 3.4us of PE work.
                # Pipeline fill (span0 only): chunk0's slot takes the
                # deferred v(ns3) chain, g==1 runs defer-1, and outproj(1)
                # splits across the g==2/g==3 slots, so every early chunk
                # has PE cover while transitioning to steady defer-2.
                if filler is not None:
                    filler()
                if g == 1 and pending:
                    emit_outproj(*pending.pop(0))
                elif g == 2 and pending:
                    warm_outb[0] = emit_outproj(*pending[0], cts=(0, 1))
                elif g == 3 and len(pending) >= 2:
                    ob0, g0 = pending.pop(0)
                    emit_outproj(ob0, g0, cts=(2, 3), outb=warm_outb[0])
                elif len(pending) >= 2:
                    emit_outproj(*pending.pop(0))

                # o = v^T @ scoresT + S^T @ qdec   [e, i] per head
                o_ps = ps.tile([P, 512], F32, tag="o", bufs=2, name=f"ops{g}")
                for h in range(H):
                    nc.tensor.matmul(
                        o_ps[:, h * L:(h + 1) * L],
                        lhsT=v_sb[:, p * 512 + h * HD: p * 512 + (h + 1) * HD],
                        rhs=scT[:, h * L:(h + 1) * L],
                        start=True, stop=False,
                    )
                    nc.tensor.matmul(
                        o_ps[:, h * L:(h + 1) * L],
                        lhsT=S_bf[:, h * HD:(h + 1) * HD],
                        rhs=q_dec[:, h * NSPAN + p * L: h * NSPAN + (p + 1) * L],
                        start=False, stop=True,
                    )
                ob = sb_pool.tile([P, 512], DT, tag="ob", bufs=3, name=f"ob{g}")
                nc.vector.tensor_copy(ob[:, 0:256], o_ps[:, 0:256])
                nc.scalar.copy(ob[:, 256:512], o_ps[:, 256:512])

                # S <- S * blockdecay + kT^T @ (kdec * v)
                su_ps = ps.tile([P, 512], F32, tag="o", bufs=2, name=f"sups{g}")
                for h in range(H):
                    nc.tensor.matmul(
                        su_ps[:, h * HD:(h + 1) * HD],
                        lhsT=kTs[:, h * 4 + p, :],
                        rhs=vdec[:, h * HD:(h + 1) * HD],
                        start=True, stop=True,
                    )
                # S decay premultiply on gpsimd (SBUF->SBUF): keeps DVE free
                nc.gpsimd.tensor_tensor(tmp_sb[:], S_bf[:], bdf_sb[:], op=mult)
                nc.vector.tensor_tensor(S_bf[:], tmp_sb[:], su_ps[:], op=add)
                pending.append((ob, g))

            for it in range(NIT):
                if 1 <= it < NIT - 1:
                    xs = xt_pool.tile([P, KT, NSPAN], DT, tag="xs", bufs=2,
                                      name=f"xs{it + 1}")
                    nc.sync.dma_start(xs[:], xd[:, it + 1, :, :])
                    xs_tiles[it + 1] = xs

                # ---- q projection, kt-major (4 parallel PSUM chains) ----
                q_ps_l = [ps.tile([P, NSPAN], F32, tag=f"qk{h}", bufs=1,
                                  name=f"qps{it}_{h}") for h in range(H)]
                for kt in range(KT):
                    for h in range(H):
                        nc.tensor.matmul(
                            q_ps_l[h][:], lhsT=wq_ap(kt, h), rhs=xts(it, kt),
                            start=(kt == 0), stop=(kt == KT - 1),
                        )
                q_raw = qk_pool.tile([P, H * NSPAN], DT, tag="q_raw", name=f"q_raw{it}")
                for h in range(H):
                    nc.vector.tensor_scalar_add(
                        q_raw[:, h * NSPAN:(h + 1) * NSPAN], q_ps_l[h][:],
                        bqk_sb[:, 2 * h:2 * h + 1])
                # q_dec from SBUF q_raw (frees the psum bank after ONE eviction)
                q_dec = qk_pool.tile([P, H * NSPAN], DT, tag="q_dec", name=f"q_dec{it}")
                for h in range(H):
                    hs = slice(h * NSPAN, (h + 1) * NSPAN)
                    nc.vector.tensor_tensor(q_dec[:, hs], q_raw[:, hs],
                                            qdec_sb[:, hs], op=mult)

                # ---- k projection, kt-major into the attn banks ----
                k_ps_l = [ps.tile([P, NSPAN], F32, tag=("sc" if h < 2 else "o"),
                                  bufs=2, name=f"kps{it}_{h}") for h in range(H)]
                for kt in range(KT):
                    for h in range(H):
                        nc.tensor.matmul(
                            k_ps_l[h][:], lhsT=wk_ap(kt, h), rhs=xts(it, kt),
                            start=(kt == 0), stop=(kt == KT - 1),
                        )
                k_sb = qk_pool.tile([P, H * NSPAN], DT, tag="k_sb", name=f"k_sb{it}")
                for h in range(H):
                    nc.scalar.activation(
                        k_sb[:, h * NSPAN:(h + 1) * NSPAN], k_ps_l[h][:],
                        IDENT, bias=bqk_sb[:, 2 * h + 1:2 * h + 2])

                # j-major k for the S updates: ONE batched XBAR dma
                # transpose of all 16 [d,128]-blocks of k_sb (~3.6us DMA,
                # one sync-ring issue) replaces 16 PE transposes (~0.85us
                # of PE per span).  kTs[j, h*4+p, d] = k_sb[d, h*512+p*128+j].
                kTs = qk_pool.tile([P, H * 4, HD], DT, tag="kTs", name=f"kTs{it}")
                nc.sync.dma_start_transpose(kTs[:], k_sb[:])


                # ---- v projection ----
                v_sb = qk_pool.tile([P, H * NSPAN], DT, tag="v_sb", name=f"v_sb{it}")

                def v_chain(ns, it=it, v_sb=v_sb):
                    vps = ps.tile([P, 512], F32, tag=f"qk{ns}", bufs=1,
                                  name=f"vps{it}_{ns}")
                    for kt in range(KT):
                        nc.tensor.matmul(
                            vps[:], lhsT=xts(it, kt)[:, ns * P:(ns + 1) * P],
                            rhs=wv_ap(kt), start=(kt == 0), stop=(kt == KT - 1),
                        )
                    nc.vector.tensor_tensor(
                        v_sb[:, ns * 512:(ns + 1) * 512], vps[:], bvf_sb[:], op=add)

                # span0 has no deferred outproj to cover chunk0's scT/kT
                # eviction latencies, so its last v chain (only needed by
                # chunk 3) moves into chunk0's outproj slot as PE filler
                for ns in range(3 if it == 0 else 4):
                    v_chain(ns)

                # ---- attention + (deferred) output projection ----
                for p in range(4):
                    filler = (lambda: v_chain(3)) if (it == 0 and p == 0) else None
                    emit_attn_chunk(it, p, q_raw, q_dec, k_sb, v_sb, kTs, filler)

            # epilogue: remaining deferred output projections, minimal tail
            emit_outproj(*pending.pop(0), mode="pen")
            emit_outproj(*pending.pop(0), mode="last")

    nc.compile()
    return nc


def _host_inputs(x, Wqkv, bqkv, Wout, bout, slopes):
    """Per-core input maps (numpy, host-side sharding + packing)."""
    in_maps = []
    # packed transpose of x, shared by the 4 cores of each batch:
    # xp[c, it, kt, n] = x[b, it*512+n, kt*128+c]
    _xtp_cache = [
        np.ascontiguousarray(
            x[b].astype(BF16).reshape(NIT, NSPAN, KT, P).transpose(3, 0, 2, 1)
        )
        for b in range(B)
    ]
    i = np.arange(L, dtype=np.float64)
    for core in range(8):
        b, g = core // 4, core % 4
        h0 = 4 * g
        hsel = slice(h0 * HD, (h0 + H) * HD)

        xb = _xtp_cache[b]

        def pack_w(Wslice):
            # (C, 512) -> [c_in_tile(128), kt*512 + col]
            return np.ascontiguousarray(
                Wslice.astype(BF16).reshape(KT, P, H * HD).transpose(1, 0, 2).reshape(P, KT * 512)
            )

        wq = pack_w(Wqkv[:, 0 * C:1 * C][:, hsel])
        wk = pack_w(Wqkv[:, 1 * C:2 * C][:, hsel])
        wv = pack_w(Wqkv[:, 2 * C:3 * C][:, hsel])
        # Wout rows for these heads: [d(128), ct*2048 + h*512 + outc]
        wo = np.ascontiguousarray(
            Wout[hsel, :].astype(BF16).reshape(H, HD, 4, 512)
            .transpose(1, 2, 0, 3).reshape(P, 4 * H * 512)
        )

        s = slopes[h0:h0 + H].astype(np.float64)  # (4,)
        diffT = (i[None, :] - i[:, None])          # [j, i] = i - j
        maskt = np.concatenate(
            [np.where(diffT >= 0, np.exp(-s[h] * diffT), 0.0) for h in range(H)],
            axis=1,
        ).astype(np.float32)                       # [128, 4*128]
        qdec_l = [np.exp(-s[h] * i) for h in range(H)]        # each (L,)
        qdec = np.concatenate(
            [np.broadcast_to(np.tile(qdec_l[h], NSPAN // L)[None, :], (P, NSPAN)) for h in range(H)],
            axis=1,
        ).astype(np.float32)                       # [128, 4*512]
        kdecv = np.concatenate(
            [np.broadcast_to(np.exp(-s[h] * (L - i))[:, None], (P, HD)) for h in range(H)],
            axis=1,
        ).astype(np.float32)                       # [128, 4*128]
        bdf = np.concatenate(
            [np.full((P, HD), math.exp(-s[h] * L)) for h in range(H)], axis=1
        ).astype(np.float32)
        # per-head, per-partition(d) q/k biases: columns [bq_h0, bk_h0, bq_h1, ...]
        bq_heads = bqkv[0 * C:1 * C][hsel].reshape(H, HD)
        bk_heads = bqkv[1 * C:2 * C][hsel].reshape(H, HD)
        bqk = np.zeros((P, 2 * H), dtype=np.float32)
        for h in range(H):
            bqk[:, 2 * h] = bq_heads[h]
            bqk[:, 2 * h + 1] = bk_heads[h]
        bvf = np.broadcast_to(bqkv[2 * C:3 * C][hsel][None, :], (P, H * HD)).astype(np.float32)

        in_maps.append({
            "x": xb, "wq": wq, "wk": wk, "wv": wv, "wo": wo,
            "maskt": maskt, "qdec": qdec, "kdecv": kdecv, "bdf": bdf,
            "bqk": bqk, "bvf": np.ascontiguousarray(bvf),
        })
    return in_maps


def kernel(x, Wqkv, bqkv, Wout, bout, slopes, _want_trace=False):
    from concourse import bass_utils

    x = np.asarray(x, dtype=np.float32)
    Wqkv = np.asarray(Wqkv, dtype=np.float32)
    bqkv = np.asarray(bqkv, dtype=np.float32)
    Wout = np.asarray(Wout, dtype=np.float32)
    bout = np.asarray(bout, dtype=np.float32)
    slopes = np.asarray(slopes, dtype=np.float32)

    if "nc" not in _CACHE:
        _CACHE["nc"] = _build()
    nc = _CACHE["nc"]

    in_maps = _host_inputs(x, Wqkv, bqkv, Wout, bout, slopes)
    res = bass_utils.run_bass_kernel_spmd(
        nc, in_maps, core_ids=list(range(8)), trace=_want_trace,
    )
    out = np.zeros((B, N, C), dtype=np.float32)
    for core in range(8):
        out[core // 4] += res.results[core]["out"].astype(np.float32)
    out += bout[None, None, :]
    if _want_trace:
        _CACHE["last_result"] = res
    return out

